# revision 65
# baseline (speedup 1.0000x reference)
import sys

sys.path.insert(0, "/opt/trn_rl_repo")

from contextlib import ExitStack

import numpy as np
import ml_dtypes
import concourse.bacc as bacc
import concourse.mybir as mybir
from concourse.bass_utils import run_bass_kernel_spmd
from concourse.tile import TileContext
from concourse.masks import make_identity

P = 128
NCORES = 8
N, D, E, KHOP, B, L = 100000, 128, 1600000, 3, 32768, 262144
H_MLP, R = 512, 64
SHARD = 12544           # nodes per core (98 * 128); core 7 padded
NP = NCORES * SHARD     # 100352
NPAIR = SHARD // 2      # 6272 pair-rows per core
NSUP = NP // 256        # 392 dst superblocks (256 nodes each)
BSEG = B // NCORES      # 4096 segments per core
BSUP = B // 256         # 128 seg superblocks
NTILE = SHARD // 256    # 49 pair tiles per core
NSUPO = NSUP // NCORES  # 49 dst superblocks per owner
SPLIT_SO = 24           # split-RS point: first 24 supers/owner go in buffer A
RANKA = NCORES * SPLIT_SO      # 192 supers in buffer A
ROWSA = RANKA * P              # 24576 pair rows in buffer A
TILEA = SPLIT_SO        # owner tiles covered by my slice of buffer A
QHOP = 8                # gsz quantum for hop schedule
QPOOL = 16              # gsz quantum for pool schedule
GB = 3072               # gather batch tokens (hops)
GBP = 1024              # gather batch tokens (pools)

f32 = mybir.dt.float32
bf16 = mybir.dt.bfloat16
i16 = mybir.dt.int16
i32 = mybir.dt.int32

_COMPILED = {}


def _wrap_idx16(idx):
    """dma_gather index layout: token i -> partition i%16, col i//16, x8 replicated."""
    n = len(idx)
    assert n % 16 == 0
    return np.tile(idx.reshape(n // 16, 16).T.astype(np.int16), (8, 1))


def _colmajor(arr):
    n = len(arr)
    assert n % P == 0
    return np.ascontiguousarray(arr.reshape(n // P, P).T)


def _schedule(bucket_by_core, slot_by_core, idx_by_core, nbuckets, quantum):
    """Static SPMD schedule for one-hot scatter matmuls.

    Tokens are sorted by bucket; each bucket's region is padded to `quantum`
    (shared across cores via max count). Chunks of 128 tokens may straddle
    bucket boundaries; straddling chunks get one matmul per bucket touched,
    with masked loc columns.

    Returns (T, mm, idx_streams, loc_tables):
      T: padded token count (mult of 128)
      mm: list of (chunk, bucket, start, stop)
      idx_streams[c]: int16 [T] gather indices (pad -> 0)
      loc_tables[c]: f32 [128, nmm] per-matmul slot columns (pad/mask -> -1)
    """
    ncores = len(bucket_by_core)
    counts = np.zeros((ncores, nbuckets), np.int64)
    for c in range(ncores):
        np.add.at(counts[c], bucket_by_core[c], 1)
    gsz = ((counts.max(axis=0) + quantum - 1) // quantum) * quantum
    # every super (bucket pair) must get at least one matmul so its partial
    # rows are written (zeros) before the reduce-scatter reads them
    for s in range(nbuckets // 2):
        if gsz[2 * s] + gsz[2 * s + 1] == 0:
            gsz[2 * s] = quantum
    starts = np.zeros(nbuckets + 1, np.int64)
    starts[1:] = np.cumsum(gsz)
    T = int(((starts[-1] + P - 1) // P) * P)

    # static matmul descriptors
    mm = []
    for b in range(nbuckets):
        if gsz[b] == 0:
            continue
        c0 = int(starts[b]) // P
        c1 = int(starts[b + 1] - 1) // P
        for ch in range(c0, c1 + 1):
            mm.append([ch, b, ch == c0, ch == c1])
    nmm = len(mm)

    idx_streams, loc_tables = [], []
    for c in range(ncores):
        order = np.argsort(bucket_by_core[c], kind="stable")
        bs = bucket_by_core[c][order]
        sl = slot_by_core[c][order].astype(np.float32)
        ix = idx_by_core[c][order].astype(np.int16)
        run_start = np.concatenate([[0], np.cumsum(counts[c])])
        pos_in_run = np.arange(len(bs)) - run_start[bs]
        out_pos = starts[bs] + pos_in_run
        idx_full = np.zeros(T, np.int16)
        idx_full[out_pos] = ix
        slot_full = np.full(T, -1.0, np.float32)
        slot_full[out_pos] = sl
        bkt_full = np.full(T, -1, np.int64)
        bkt_full[out_pos] = bs
        # build per-mm loc columns: tokens of chunk ch masked to bucket b
        loc_cols = np.full((nmm, P), -1.0, np.float32)
        for j, (ch, b, _s, _e) in enumerate(mm):
            tok = slice(ch * P, ch * P + P)
            m = bkt_full[tok] == b
            col = np.full(P, -1.0, np.float32)
            col[m] = slot_full[tok][m]
            loc_cols[j] = col
        idx_streams.append(idx_full)
        loc_tables.append(np.ascontiguousarray(loc_cols.T))  # [128, nmm]
    return T, mm, idx_streams, loc_tables


def _build_program(TH, mmH, nmmPh, mmPh, nmmPt, mmPt, TPh, TPt):
    nc = bacc.Bacc("TRN2", target_bir_lowering=False, num_devices=NCORES)

    nmmH = len(mmH)

    # ---------------- dram tensors ----------------
    embed_in = nc.dram_tensor("embed_in", [NPAIR, 2 * D], f32, kind="ExternalInput")
    degs_in = nc.dram_tensor("degs_in", [P, NTILE, 2], i32, kind="ExternalInput")
    degd_in = nc.dram_tensor("degd_in", [P, NTILE, 2], i32, kind="ExternalInput")
    temp_in = nc.dram_tensor("temp_in", [P, 4], f32, kind="ExternalInput")
    wrep_in = nc.dram_tensor("wrep_in", [P, 2 * D], f32, kind="ExternalInput")
    w1_in = nc.dram_tensor("w1_in", [3, P, H_MLP], bf16, kind="ExternalInput")
    b1_in = nc.dram_tensor("b1_in", [P, 4], f32, kind="ExternalInput")
    w2_in = nc.dram_tensor("w2_in", [4, P, R], bf16, kind="ExternalInput")
    b2_in = nc.dram_tensor("b2_in", [R, 1], f32, kind="ExternalInput")
    hsrc = nc.dram_tensor("hsrc", [P, TH // 16], i16, kind="ExternalInput")
    hloc = nc.dram_tensor("hloc", [P, nmmH], f32, kind="ExternalInput")
    psrcH = nc.dram_tensor("psrcH", [P, TPh // 16], i16, kind="ExternalInput")
    plocH = nc.dram_tensor("plocH", [P, nmmPh], f32, kind="ExternalInput")
    psrcT = nc.dram_tensor("psrcT", [P, TPt // 16], i16, kind="ExternalInput")
    plocT = nc.dram_tensor("plocT", [P, nmmPt], f32, kind="ExternalInput")

    out = nc.dram_tensor("out", [BSEG, R], f32, kind="ExternalOutput")

    xloc_a = nc.dram_tensor("xloc_a", [SHARD, D], bf16)
    xloc_b = nc.dram_tensor("xloc_b", [SHARD, D], bf16)
    f8 = mybir.dt.float8e4
    xparts = [nc.dram_tensor("xpart0", [NP // 2, 2 * D], bf16),
              nc.dram_tensor("xpart1", [NP // 2, 2 * D], f8),
              nc.dram_tensor("xpart2", [NP // 2, 2 * D], f8)]
    xnews = [nc.dram_tensor("xnew0", [NPAIR, 2 * D], bf16),
             nc.dram_tensor("xnew1", [NPAIR, 2 * D], f8),
             nc.dram_tensor("xnew2", [NPAIR, 2 * D], f8)]
    zext = nc.dram_tensor("zext", [SHARD, 2 * D], bf16)
    ppart = nc.dram_tensor("ppart", [B, 260], bf16)
    pout = nc.dram_tensor("pout", [BSEG, 260], bf16)

    rg = [list(range(NCORES))]

    with TileContext(nc) as tc, ExitStack() as ctx:
        sb = ctx.enter_context(tc.tile_pool(name="sb", bufs=2))
        const = ctx.enter_context(tc.tile_pool(name="const", bufs=1))
        gpool = ctx.enter_context(tc.tile_pool(name="gath", bufs=4))
        gppool = ctx.enter_context(tc.tile_pool(name="gathp", bufs=2))
        ohp = ctx.enter_context(tc.tile_pool(name="ohp", bufs=2))
        ccs = ctx.enter_context(nc.semaphore("ccs"))
        ccs_val = [0]

        def coll(kind, ins_ap, outs_ap):
            with tc.tile_critical():
                ccs_val[0] += 1
                op = mybir.AluOpType.bypass if kind == "AllGather" \
                    else mybir.AluOpType.add
                nc.gpsimd.collective_compute(
                    kind, op, ins=[ins_ap], outs=[outs_ap], replica_groups=rg,
                ).then_inc(ccs, 1)
                nc.gpsimd.wait_ge(ccs, ccs_val[0])

        # hop gather index/loc tables first: hop-1 gathers depend on them
        hop_idx = const.tile([P, TH // 16], i16)
        nc.sync.dma_start(hop_idx[:], hsrc[:])
        hop_loc = const.tile([P, nmmH], f32)
        nc.sync.dma_start(hop_loc[:], hloc[:])

        # ---------------- constants ----------------
        iota_i = const.tile([P, P], i32)
        nc.gpsimd.iota(iota_i[:], pattern=[[1, P]], base=0, channel_multiplier=0)
        iota_b = const.tile([P, P], bf16)
        nc.vector.tensor_copy(iota_b[:], iota_i[:])
        ident = const.tile([P, P], f32)
        make_identity(nc, ident[:])

        temp_sb = const.tile([P, 4], f32)
        nc.sync.dma_start(temp_sb[:], temp_in[:])
        wrep = const.tile([P, 2 * D], f32)
        nc.sync.dma_start(wrep[:], wrep_in[:])
        w1t = const.tile([P, 3, H_MLP], bf16)
        nc.sync.dma_start(w1t[:], w1_in.rearrange("k p h -> p k h")[:])
        b1t = const.tile([P, 4], f32)
        nc.sync.dma_start(b1t[:], b1_in[:])
        w2t = const.tile([P, 4, R], bf16)
        nc.sync.dma_start(w2t[:], w2_in.rearrange("k p r -> p k r")[:])
        b2t = const.tile([R, 1], f32)
        nc.sync.dma_start(b2t[:], b2_in[:])

        # ---------------- degree scales (pair layout [p, t, h]) ----------------
        def rsqrt_deg(deg_ap, tag):
            di = sb.tile([P, NTILE, 2], i32, tag="degi")
            nc.sync.dma_start(di[:], deg_ap)
            df = sb.tile([P, NTILE, 2], f32, tag="degf")
            nc.vector.tensor_copy(df[:], di[:])
            dm = sb.tile([P, NTILE, 2], f32, tag="degm")
            nc.vector.tensor_scalar(out=dm[:], in0=df[:], scalar1=1.0,
                                    scalar2=None, op0=mybir.AluOpType.max)
            rec = sb.tile([P, NTILE, 2], f32, tag="degr")
            nc.vector.reciprocal(rec[:], dm[:])
            res = const.tile([P, NTILE, 2], f32, tag=tag)
            nc.scalar.activation(res[:], rec[:],
                                 mybir.ActivationFunctionType.Sqrt)
            return res

        a_sc = rsqrt_deg(degs_in[:], "a_sc")      # rsqrt(deg_src) of my nodes
        b_sc = rsqrt_deg(degd_in[:], "b_sc")      # rsqrt(deg_dst) of my nodes
        absc = const.tile([P, NTILE, 2], f32)
        nc.vector.tensor_tensor(out=absc[:], in0=a_sc[:], in1=b_sc[:],
                                op=mybir.AluOpType.mult)
        btsc = const.tile([P, 3, NTILE, 2], f32)
        for k in range(3):
            nc.vector.tensor_scalar(out=btsc[:, k, :, :], in0=b_sc[:],
                                    scalar1=temp_sb[:, k + 1:k + 2], scalar2=None,
                                    op0=mybir.AluOpType.mult)

        # ---------------- init: hidden = temp0*embed ; xloc = a_sc*embed -------
        hidden = const.tile([P, NTILE, 2 * D], f32)
        nc.sync.dma_start(hidden[:], embed_in.rearrange("(t p) d -> p t d", p=P)[:])
        for t in range(NTILE):
            xw = sb.tile([P, 2 * D], bf16, tag="xw")
            for h in range(2):
                nc.any.tensor_scalar(out=xw[:, h * D:(h + 1) * D],
                                     in0=hidden[:, t, h * D:(h + 1) * D],
                                     scalar1=a_sc[:, t, h:h + 1], scalar2=None,
                                     op0=mybir.AluOpType.mult)
            nc.sync.dma_start(
                xloc_a.rearrange("(t p2 h) d -> p2 t (h d)", p2=P, h=2)[:, t, :],
                xw[:])
        nc.vector.tensor_scalar(out=hidden[:], in0=hidden[:],
                                scalar1=temp_sb[:, 0:1], scalar2=None,
                                op0=mybir.AluOpType.mult)

        # ---------------- hops ----------------
        # mmH entries: (chunk, bucket, start, stop); bucket = super*2 + parity
        # precompute per-super last mm index
        last_mm_of_super = {}
        for j, (ch, b, s0, s1) in enumerate(mmH):
            last_mm_of_super[b // 2] = j

        EG = 4  # supers per evac DMA group

        def psum_copy(dst_ap, src_ap, which):
            if which % 2 == 0:
                nc.scalar.activation(dst_ap, src_ap,
                                     mybir.ActivationFunctionType.Copy)
            else:
                nc.vector.tensor_copy(dst_ap, src_ap)

        nbatch = (TH + GB - 1) // GB
        PF = 3  # gather prefetch depth

        def hop_update(k, xnew, evdt, t_lo, t_hi, wait_val):
            """Owner update for tiles [t_lo:t_hi): hidden += btsc*xnew and
            (except last hop) xloc_next = absc*xnew."""
            xdst = xloc_b if k % 2 == 0 else xloc_a
            for t0 in range(t_lo, t_hi, 4):
                nt = min(4, t_hi - t0)
                xn4 = sb.tile([P, 4, 2 * D], evdt, tag="xn4k" + str(k))
                nc.sync.dma_start(
                    xn4[:, 0:nt, :],
                    xnew.rearrange("(t p) d -> p t d", p=P)[:, t0:t0 + nt, :])
                xw4 = sb.tile([P, 4, 2 * D], bf16, tag="xw4")
                for t in range(t0, t0 + nt):
                    for h in range(2):
                        if k < KHOP - 1:
                            nc.any.tensor_scalar(
                                out=xw4[:, t - t0, h * D:(h + 1) * D],
                                in0=xn4[:, t - t0, h * D:(h + 1) * D],
                                scalar1=absc[:, t, h:h + 1], scalar2=None,
                                op0=mybir.AluOpType.mult)
                        tmp = sb.tile([P, D], f32, tag="tmp")
                        nc.any.tensor_scalar(
                            out=tmp[:], in0=xn4[:, t - t0, h * D:(h + 1) * D],
                            scalar1=btsc[:, k, t, h:h + 1], scalar2=None,
                            op0=mybir.AluOpType.mult)
                        nc.any.tensor_tensor(
                            out=hidden[:, t, h * D:(h + 1) * D],
                            in0=hidden[:, t, h * D:(h + 1) * D], in1=tmp[:],
                            op=mybir.AluOpType.add)
                if k < KHOP - 1:
                    nc.sync.dma_start(
                        xdst.rearrange("(t p2 h) d -> p2 t (h d)",
                                       p2=P, h=2)[:, t0:t0 + nt, :],
                        xw4[:, 0:nt, :])

        with tc.tile_pool(name="psh", bufs=2, space="PSUM") as psh:
            for k in range(KHOP):
                xsrc = xloc_a if k % 2 == 0 else xloc_b
                xpart = xparts[k]
                xnew = xnews[k]
                evdt = bf16 if k == 0 else mybir.dt.float8e4

                gtiles = [None] * nbatch
                emitted = [0]

                def need_batch(bi, k=k, xsrc=xsrc, gtiles=gtiles, emitted=emitted):
                    while emitted[0] <= min(bi + PF, nbatch - 1):
                        b_ = emitted[0]
                        t0 = b_ * GB
                        n_ = min(GB, TH - t0)
                        gt = gpool.tile([P, GB // P, D], bf16, tag="gt",
                                        name="gt")
                        gtiles[b_] = gt
                        nc.gpsimd.dma_gather(
                            gt[:, 0:n_ // P, :], xsrc[:],
                            hop_idx[:, t0 // 16:(t0 + n_) // 16],
                            n_, n_, D, single_packet=False)
                        emitted[0] += 1

                # matmuls with just-in-time gathers and mid-stream split RS
                ps = {}
                ev4 = None
                val_a = None
                for j, (ch, b, st, sp) in enumerate(mmH):
                    need_batch(ch // (GB // P))
                    oh_t = ohp.tile([P, P], bf16, tag="oh", bufs=8)
                    nc.vector.tensor_scalar(
                        out=oh_t[:], in0=iota_b[:],
                        scalar1=hop_loc[:, j:j + 1], scalar2=None,
                        op0=mybir.AluOpType.is_equal)
                    sup, h = b // 2, b % 2
                    if st:
                        ps[b] = psh.tile([P, D], f32, tag=f"ps{h}{sup % 2}",
                                         name=f"psh_{h}_{sup % 2}")
                    nc.tensor.matmul(
                        ps[b][:], lhsT=oh_t[:],
                        rhs=gtiles[ch // (GB // P)][:, ch % (GB // P), :],
                        start=st, stop=sp)
                    if sp and last_mm_of_super.get(sup) == j:
                        gi = sup % EG
                        if gi == 0:
                            ev4 = sb.tile([P, EG, 2 * D], evdt, tag="ev4k" + str(k))
                        for h2 in range(2):
                            bb = sup * 2 + h2
                            dst = ev4[:, gi, h2 * D:(h2 + 1) * D]
                            if bb in ps:
                                psum_copy(dst, ps[bb][:], sup * 2 + h2)
                                del ps[bb]
                            else:
                                nc.vector.memset(dst, 0.0)
                        if gi == EG - 1 or sup == NSUP - 1:
                            g = sup // EG
                            nc.sync.dma_start(
                                xpart.rearrange("(g gi p2) d -> p2 g gi d",
                                                gi=EG, p2=P)[:, g, 0:gi + 1, :],
                                ev4[:, 0:gi + 1, :])
                coll("ReduceScatter", xpart[:], xnew[:])
                hop_update(k, xnew, evdt, 0, NTILE, 0)

        # ---------------- z_ext = [z | 1 | s | 0...] per node ----------------
        for t0 in range(0, NTILE, 4):
            nt = min(4, NTILE - t0)
            ze4 = sb.tile([P, 4, 2, 2 * D], bf16, tag="ze4")
            for t in range(t0, t0 + nt):
                ti = t - t0
                prod = sb.tile([P, 2 * D], f32, tag="prod")
                nc.vector.tensor_tensor(out=prod[:], in0=hidden[:, t, :],
                                        in1=wrep[:], op=mybir.AluOpType.mult)
                s2 = sb.tile([P, 2, 1], f32, tag="s2")
                nc.vector.reduce_sum(
                    s2[:], prod[:].rearrange("p (h d) -> p h d", h=2),
                    axis=mybir.AxisListType.X)
                for h in range(2):
                    nc.scalar.activation(ze4[:, ti, h, 0:D],
                                         hidden[:, t, h * D:(h + 1) * D],
                                         mybir.ActivationFunctionType.Copy)
                nc.gpsimd.memset(ze4[:, ti, :, D:], 0.0)
                nc.gpsimd.memset(ze4[:, ti, :, D:D + 1], 1.0)
                nc.gpsimd.tensor_copy(ze4[:, ti, :, D + 1:D + 2], s2[:])
            nc.sync.dma_start(
                zext.rearrange("(t p2 h) d -> p2 t (h d)",
                               p2=P, h=2)[:, t0:t0 + nt, :],
                ze4[:, 0:nt, :].rearrange("p n h d -> p n (h d)"))

        # ---------------- pooling (owner-sharded partials) ----------------
        pool_idx_h = const.tile([P, TPh // 16], i16)
        nc.sync.dma_start(pool_idx_h[:], psrcH[:])
        pool_loc_h = const.tile([P, nmmPh], f32)
        nc.sync.dma_start(pool_loc_h[:], plocH[:])
        pool_idx_t = const.tile([P, TPt // 16], i16)
        nc.sync.dma_start(pool_idx_t[:], psrcT[:])
        pool_loc_t = const.tile([P, nmmPt], f32)
        nc.sync.dma_start(pool_loc_t[:], plocT[:])

        def pool_stream(TP, idx_t, tag):
            """Emit gathers for one pool stream; return (gtiles, ecol)."""
            nbatch = (TP + GBP - 1) // GBP
            gtiles = []
            for bi in range(nbatch):
                t0 = bi * GBP
                n_ = min(GBP, TP - t0)
                gt = gppool.tile([P, GBP // P, 2 * D], bf16, tag="gtp" + tag,
                                 name="gt" + tag)
                gtiles.append(gt)
                nc.gpsimd.dma_gather(
                    gt[:, 0:n_ // P, :], zext[:],
                    idx_t[:, t0 // 16:(t0 + n_) // 16],
                    n_, n_, 2 * D, single_packet=False)
            nch = TP // P
            ecol = const.tile([P, nch], f32, tag="ecol" + tag, name="ecol" + tag)
            for bi in range(nbatch):
                c0 = bi * (GBP // P)
                nb_ = min(GBP // P, nch - c0)
                nc.scalar.activation(
                    ecol[:, c0:c0 + nb_].unsqueeze(2),
                    gtiles[bi][:, 0:nb_, D + 1:D + 2],
                    mybir.ActivationFunctionType.Exp)
            return gtiles, ecol

        def by_super(mmP):
            d = {}
            for j, (ch, b, st, sp) in enumerate(mmP):
                d.setdefault(b // 2, []).append((j, ch, b, st, sp))
            return d

        with tc.tile_pool(name="psp", bufs=1, space="PSUM") as psp:
            gtH, ecolH = pool_stream(TPh, pool_idx_h, "H")
            gtT, ecolT = pool_stream(TPt, pool_idx_t, "T")
            supH, supT = by_super(mmPh), by_super(mmPt)
            ev2 = None
            nev = 0
            for sup in range(BSUP):
                gi = sup % 2
                if gi == 0:
                    ev2 = sb.tile([P, 4, 260], bf16, tag="evp")
                for (pn, sups, gts, ecol, loc_t, c0) in (
                        ("H", supH, gtH, ecolH, pool_loc_h, 0),
                        ("T", supT, gtT, ecolT, pool_loc_t, 130)):
                    ps = {}
                    for (j, ch, b, st, sp) in sups.get(sup, []):
                        ohw_t = ohp.tile([P, P], bf16, tag="ohwp", bufs=8,
                                         name="ohw")
                        nc.vector.tensor_scalar(
                            out=ohw_t[:], in0=iota_b[:],
                            scalar1=loc_t[:, j:j + 1],
                            scalar2=ecol[:, ch:ch + 1],
                            op0=mybir.AluOpType.is_equal,
                            op1=mybir.AluOpType.mult)
                        h = b % 2
                        if st:
                            ps[h] = psp.tile([P, 256], f32,
                                             tag=f"pp{pn}{h}{sup % 2}",
                                             name=f"psp{pn}{h}{sup % 2}")
                        nc.tensor.matmul(
                            ps[h][:, 0:130], lhsT=ohw_t[:],
                            rhs=gts[ch // (GBP // P)][:, ch % (GBP // P), 0:130],
                            start=st, stop=sp)
                    for h2 in range(2):
                        dst = ev2[:, gi * 2 + h2, c0:c0 + 130]
                        if h2 in ps:
                            psum_copy(dst, ps[h2][:, 0:130], nev)
                            nev += 1
                        else:
                            nc.vector.memset(dst, 0.0)
                if gi == 1 or sup == BSUP - 1:
                    g = sup // 2
                    nc.sync.dma_start(
                        ppart.rearrange("(g q p2) d -> p2 g q d",
                                        q=4, p2=P)[:, g, :, :],
                        ev2[:])

        coll("ReduceScatter", ppart[:], pout[:])

        # ---------------- normalize + MLP (my 4096 segments) ----------------
        NSB = BSEG // P  # 32
        with tc.tile_pool(name="psm", bufs=2, space="PSUM") as psm:
            po4 = None
            lt4 = None
            for t in range(NSB):
                if t % 4 == 0:
                    po4 = sb.tile([P, 4, 260], bf16, tag="po4")
                    nc.sync.dma_start(
                        po4[:],
                        pout.rearrange("(t p) d -> p t d", p=P)[:, t:t + 4, :])
                    lt4 = sb.tile([P, 4, R], f32, tag="lt4")
                feats = []
                fts = []
                for (c0, tagf) in ((0, "fh"), (130, "ft")):
                    den = sb.tile([P, 1], f32, tag="den" + tagf)
                    nc.vector.tensor_scalar(out=den[:],
                                            in0=po4[:, t % 4, c0 + D:c0 + D + 1],
                                            scalar1=1e-30, scalar2=None,
                                            op0=mybir.AluOpType.max)
                    rden = sb.tile([P, 1], f32, tag="rden" + tagf)
                    nc.vector.reciprocal(rden[:], den[:])
                    pooled = sb.tile([P, D], f32, tag="pl" + tagf)
                    nc.any.tensor_scalar(out=pooled[:],
                                         in0=po4[:, t % 4, c0:c0 + D],
                                         scalar1=rden[:], scalar2=None,
                                         op0=mybir.AluOpType.mult)
                    pt = psm.tile([P, P], f32, tag="pt")
                    nc.tensor.transpose(out=pt[:], in_=pooled[:], identity=ident[:])
                    ft = sb.tile([P, P], bf16, tag="ftr" + tagf)
                    nc.vector.tensor_copy(ft[:], pt[:])
                    fts.append(ft)
                htT = sb.tile([P, P], bf16, tag="htT")
                nc.any.tensor_tensor(out=htT[:], in0=fts[0][:], in1=fts[1][:],
                                     op=mybir.AluOpType.mult)
                feats = [fts[0], fts[1], htT]

                o1 = sb.tile([P, 4, P], bf16, tag="o1")
                for m in range(4):
                    ps1 = psm.tile([P, P], f32, tag="ps1")
                    for kk in range(3):
                        nc.tensor.matmul(ps1[:],
                                         lhsT=w1t[:, kk, m * P:(m + 1) * P],
                                         rhs=feats[kk][:],
                                         start=kk == 0, stop=kk == 2)
                    nc.scalar.activation(o1[:, m, :], ps1[:],
                                         mybir.ActivationFunctionType.Relu,
                                         bias=b1t[:, m:m + 1])
                ps2 = psm.tile([R, P], f32, tag="ps2", padded_shape=[P, P])
                for kk in range(4):
                    nc.tensor.matmul(ps2[:], lhsT=w2t[:, kk, :], rhs=o1[:, kk, :],
                                     start=kk == 0, stop=kk == 3)
                lg = sb.tile([R, P], f32, tag="lg")
                nc.vector.tensor_scalar(out=lg[:], in0=ps2[:], scalar1=b2t[:],
                                        scalar2=None, op0=mybir.AluOpType.add)
                lt = psm.tile([P, R], f32, tag="lt", padded_shape=[P, P])
                nc.tensor.transpose(out=lt[:], in_=lg[:], identity=ident[:R, :R])
                nc.vector.tensor_copy(lt4[:, t % 4, :], lt[:])
                if t % 4 == 3:
                    nc.sync.dma_start(
                        out.rearrange("(t p) r -> p t r", p=P)[:, t - 3:t + 1, :],
                        lt4[:])

    nc.compile()
    return nc


def kernel(embed, temp, attn_w, attn_b, W1, b1, W2, b2,
           edge_index, H_idx, H_seg, T_idx, T_seg, B):
    embed = np.asarray(embed, np.float32)
    temp = np.asarray(temp, np.float32)
    attn_w = np.asarray(attn_w, np.float32)
    W1 = np.asarray(W1, np.float32)
    b1 = np.asarray(b1, np.float32)
    W2 = np.asarray(W2, np.float32)
    b2 = np.asarray(b2, np.float32)
    edge_index = np.asarray(edge_index)
    H_idx, H_seg = np.asarray(H_idx), np.asarray(H_seg)
    T_idx, T_seg = np.asarray(T_idx), np.asarray(T_seg)

    src = edge_index[0].astype(np.int64)
    dst = edge_index[1].astype(np.int64)

    # hop schedule: edges owned by src shard; superblocks processed in
    # (so < SPLIT_SO, owner, so) order so the first RANKA supers form a
    # contiguous partial buffer that can reduce-scatter early (split-RS).
    rank_of_sup = np.arange(NSUP)
    hb, hs, hi = [], [], []
    for c in range(NCORES):
        lo = c * SHARD
        m = (src >= lo) & (src < lo + SHARD)
        d = dst[m]
        hb.append((rank_of_sup[d >> 8] * 2 + (d & 1)).astype(np.int64))
        hs.append((d >> 1) & 127)
        hi.append(src[m] - lo)
    TH, mmH, hidxs, hlocs = _schedule(hb, hs, hi, NSUP * 2, QHOP)

    # pool schedules: entries owned by idx shard; bucket = seg super * 2 + parity
    def pool_sched(idx, seg):
        pb, psl, pii = [], [], []
        idx = idx.astype(np.int64)
        seg = seg.astype(np.int64)
        for c in range(NCORES):
            lo = c * SHARD
            m = (idx >= lo) & (idx < lo + SHARD)
            sg = seg[m]
            pb.append(((sg >> 8) * 2 + ((sg >> 7) & 1)).astype(np.int64))
            psl.append(sg & 127)
            pii.append(idx[m] - lo)
        return _schedule(pb, psl, pii, BSUP * 2, QPOOL)

    TPh, mmPh, pidxsH, plocsH = pool_sched(H_idx, H_seg)
    TPt, mmPt, pidxsT, plocsT = pool_sched(T_idx, T_seg)

    key = (TH, len(mmH), TPh, len(mmPh), TPt, len(mmPt))
    if key not in _COMPILED:
        _COMPILED[key] = _build_program(TH, mmH, len(mmPh), mmPh,
                                        len(mmPt), mmPt, TPh, TPt)
    nc = _COMPILED[key]

    deg_dst = np.bincount(dst, minlength=NP).astype(np.int32)
    deg_src = np.bincount(src, minlength=NP).astype(np.int32)

    def pair_layout_i32(v, c):
        # [p, t, h] for nodes (t*128+p)*2+h of core c
        lo = c * SHARD
        arr = v[lo:lo + SHARD].reshape(NTILE, P, 2)  # [(t p h)] -> t, p, h
        return np.ascontiguousarray(arr.transpose(1, 0, 2))

    bf = ml_dtypes.bfloat16
    in_maps = []
    for c in range(NCORES):
        lo = c * SHARD
        n_real = max(0, min(SHARD, N - lo))
        esh = np.zeros((SHARD, D), np.float32)
        esh[:n_real] = embed[lo:lo + n_real]
        wr = np.tile(attn_w[:, 0][None, :], (P, 2))
        in_maps.append(dict(
            embed_in=esh.reshape(NPAIR, 2 * D),
            degs_in=pair_layout_i32(deg_src, c),
            degd_in=pair_layout_i32(deg_dst, c),
            temp_in=np.tile(temp[None, :], (P, 1)),
            wrep_in=wr,
            w1_in=W1.reshape(3, P, H_MLP).astype(bf),
            b1_in=np.ascontiguousarray(b1.reshape(4, P).T),
            w2_in=W2.reshape(4, P, R).astype(bf),
            b2_in=b2[:, None].copy(),
            hsrc=_wrap_idx16(hidxs[c]),
            hloc=hlocs[c],
            psrcH=_wrap_idx16(pidxsH[c]),
            plocH=plocsH[c],
            psrcT=_wrap_idx16(pidxsT[c]),
            plocT=plocsT[c],
        ))

    res = run_bass_kernel_spmd(nc, in_maps, list(range(NCORES)))
    return np.concatenate([res.results[c]["out"] for c in range(NCORES)], axis=0)
